# revision 49
# baseline (speedup 1.0000x reference)
"""GAT-style message passing (edge softmax + weighted aggregation) on 8 NeuronCores.

In-situ gather-GEMM design (v3):
  - Per edge slot, the raw bf16 feature row of the source node is gathered
    directly from HBM with a transposed MoE gather (dma_gather transpose=True),
    landing in matmul-lhsT layout [din%128, din//128, slot].  Gather calls are
    chunked 768/768/512 per half-group (the ucode SWDGE ring caps a transpose
    call below 1024 idxs; 2 descriptors per idx).
  - Narrow blocks: <= 32 dsts x 4 tiles (512 slots, 2 per src-half), halving
    the one-hot S / S^T HBM traffic of 64-dst blocks.
  - Split GEMM: hs (128 cols) accumulates into a per-block 1-bank PSUM tile;
    el (8 cols) goes to a shared per-sub-batch PSUM tile, and the er one-hot
    expansion matmul accumulates er+c into the SAME el columns, so
    x = el + er + c comes out of the PE with no vector add.
  - w = max(exp(x), exp(0.2 x)) with both exps reading el PSUM directly
    (leakyrelu+exp fused via exp-of-max; softmax max-subtraction dropped:
    logits are O(1)).  Hybrid PSUM drain: K_MF blocks per sub-batch get a
    merged DVE move+multiply (zsb.hs = hs_psum * w); the rest are plain Act
    copies finished by one in-place 2x-rate bf16 DVE multiply.  w lands in
    zsb's el columns so the segsum denominator rides along as 8 extra cols.
  - er per destination: the host stages pre-transposed, per-pair-padded dst
    feature rows (fdp) so a plain DMA + tiny GEMM against W_er produces er
    for 16 blocks at a time (no SWDGE call), with c = c_el + c_er folded in
    on the PSUM->SBUF move.
  - Segment-sum as matmul with a host-built fp8 one-hot S per tile; stg /
    normalize / bias all in bf16 with packed head-minor layouts (DVE 2x),
    output written per group as a contiguous padded bf16 [nb*32, 128] table
    in head-minor channel order; the host unpads, unpermutes and upcasts.
  - The last gather-group runs 4 half-size sub-batches on alternating PSUM
    pool pairs to shorten the drain tail.

Edges are sorted by dst; dst nodes split into 8 contiguous per-core ranges
with ~equal edge counts; consecutive dsts greedily packed into blocks of
<= 32 dsts and <= 2*128 edge slots per src-half (src < 25000 goes to the A
half so gather indices stay non-negative int16 — the SWDGE ucode rejects
negative indices, probed on HW).
"""

import sys

for _p in ("/opt/trn_rl_repo",):
    if _p not in sys.path:
        sys.path.insert(0, _p)

import os

import numpy as np
import ml_dtypes

DBG_NBG = int(os.environ.get("K_NBG_LIMIT", "0"))  # truncate groups if >0
DBG_CORES = int(os.environ.get("K_CORES", "0"))  # run on fewer cores if >0
K_SCRATCH = int(os.environ.get("K_SCRATCH", "16384"))  # swdge ring bytes
K_SUBS = tuple(
    int(x) for x in os.environ.get("K_SUBS", "768,768").split(",")
)  # gather chunk sizes per half-group
K_MF = int(os.environ.get("K_MF", "1"))  # merged-msb blocks per sub-batch
K_NQ = int(os.environ.get("K_NQ", "1"))  # swdge queues (gathers alternate)

import concourse.bass as bass
import concourse.bacc as bacc
import concourse.mybir as mybir
import concourse.tile as tile
from concourse.bass_utils import run_bass_kernel_spmd

BF16 = ml_dtypes.bfloat16
FP8 = ml_dtypes.float8_e4m3
P = 128


class Cfg:
    def __init__(self, n_nodes, d_in, kh, dh, n_cores, sw, tpb, bg, sub, neg_slope=0.2):
        assert d_in % P == 0
        self.n_nodes = n_nodes
        self.d_in = d_in
        self.kh = kh
        self.dh = dh
        self.c = kh * dh  # 128
        self.n_cores = n_cores
        self.sw = sw  # dsts per block
        self.tpb = tpb  # tiles (128 slots) per block
        self.bg = bg  # blocks per gather-group
        self.sub = sub  # blocks per compute sub-batch
        self.neg_slope = neg_slope
        self.kc = d_in // P
        self.zgc = self.c + kh  # 136: hs + el
        self.split = n_nodes // 2
        self.tpa = tpb // 2  # A-half tiles per block
        self.rng_pad = 6400  # uniform local-dst table size (>= max core range)
        assert tpb % 2 == 0 and bg % sub == 0 and bg % 2 == 0
        assert self.split <= 32768 and (n_nodes - self.split) <= 32768
        assert self.c == P


FULL_CFG = Cfg(
    n_nodes=50000, d_in=256, kh=8, dh=16, n_cores=8, sw=32, tpb=4, bg=6, sub=3
)


# ----------------------------------------------------------------------------
# Host-side preprocessing
# ----------------------------------------------------------------------------

def _head_minor_perm(cfg):
    c = np.arange(cfg.c)
    return (c % cfg.kh) * cfg.dh + (c // cfg.kh)


def build_weights(cfg, W_src, b_src, W_dst, b_dst, attn):
    kh, dh, din = cfg.kh, cfg.dh, cfg.d_in
    a_l = attn[:, :dh]
    a_r = attn[:, dh:]
    perm = _head_minor_perm(cfg)

    W_el = np.einsum("kd,kdi->ki", a_l, W_src.reshape(kh, dh, din))  # (kh, din)
    W_er = np.einsum("kd,kdi->ki", a_r, W_dst.reshape(kh, dh, din))
    c_el = np.einsum("kd,kd->k", a_l, b_src.reshape(kh, dh))
    c_er = np.einsum("kd,kd->k", a_r, b_dst.reshape(kh, dh))

    Wz = np.concatenate([W_src[perm].T, W_el.T], axis=1)  # (din, zgc)
    Wz = np.ascontiguousarray(
        Wz.reshape(cfg.kc, P, cfg.zgc).transpose(1, 0, 2)
    ).astype(BF16)  # (P, kc, zgc)
    Wer = np.ascontiguousarray(
        W_er.T.reshape(cfg.kc, P, kh).transpose(1, 0, 2)
    ).astype(BF16)  # (P, kc, kh)

    c_vec = (c_el + c_er).astype(np.float32)
    c_rep = np.tile(c_vec[None, :], (P, 1)).astype(BF16)  # (P, kh)
    b_rep = np.tile(b_src[perm][None, :], (cfg.sw, 1)).astype(BF16)  # (sw, c) head-minor
    return Wz, Wer, c_rep, b_rep


def build_schedule(cfg, src_idx, dst_idx):
    """Sort edges by dst, split dsts across cores, pack blocks, build per-core
    index / one-hot / er-offset arrays (uniform shapes across cores)."""
    E = src_idx.shape[0]
    n = cfg.n_nodes
    order = np.argsort(dst_idx, kind="stable")
    ssrc = src_idx[order].astype(np.int64)
    counts = np.bincount(dst_idx, minlength=n).astype(np.int64)
    starts = np.zeros(n + 1, dtype=np.int64)
    np.cumsum(counts, out=starts[1:])
    acnt = np.bincount(dst_idx[src_idx < cfg.split], minlength=n).astype(np.int64)

    bounds = [0]
    for ci in range(1, cfg.n_cores):
        target = E * ci // cfg.n_cores
        d = int(np.searchsorted(starts, target))
        d = max(bounds[-1], min(d, n))
        bounds.append(d)
    bounds.append(n)

    per_core_blocks = []  # list of list of (d0, ndst, edge_start)
    cap = cfg.tpa * P
    for ci in range(cfg.n_cores):
        d0, d1 = bounds[ci], bounds[ci + 1]
        assert d1 - d0 < cfg.rng_pad
        blocks = []
        d = d0
        while d < d1:
            bstart = d
            ua = ub = 0
            while (
                d < d1
                and (d - bstart) < cfg.sw
                and ua + acnt[d] <= cap
                and ub + (counts[d] - acnt[d]) <= cap
            ):
                ua += int(acnt[d])
                ub += int(counts[d] - acnt[d])
                d += 1
            assert d > bstart, f"dst {d} degree {counts[d]} exceeds block caps"
            blocks.append((bstart, d - bstart, int(starts[bstart])))
        per_core_blocks.append(blocks)

    nb_real = max(len(b) for b in per_core_blocks)
    nbg = -(-nb_real // cfg.bg)
    nb = nbg * cfg.bg

    nia = cfg.bg * cfg.tpa * P  # gather idxs per half-group
    trash = cfg.rng_pad - 1
    idxa = np.zeros((cfg.n_cores, nbg, nia), dtype=np.int16)
    idxb = np.zeros((cfg.n_cores, nbg, nia), dtype=np.int16)
    s_arr = np.zeros((cfg.n_cores, nb, cfg.tpb, P, cfg.sw), dtype=np.float32)
    dloc_b = np.full((cfg.n_cores, nbg, P, cfg.bg * cfg.tpb), 255.0, dtype=BF16)
    st2 = np.zeros((cfg.n_cores, nbg, cfg.sw, cfg.bg * cfg.tpb * P), dtype=np.float32)
    # er dst-local indices: per group, bg*sw rows
    gdl = np.full((cfg.n_cores, nbg, cfg.bg * cfg.sw), trash, dtype=np.int16)

    for ci in range(cfg.n_cores):
        d0c = bounds[ci]
        for bi, (bstart, ndst, estart) in enumerate(per_core_blocks[ci]):
            gi, bj = bi // cfg.bg, bi % cfg.bg
            nedges = int(starts[bstart + ndst] - starts[bstart])
            if nedges > 0:
                esrc = ssrc[estart : estart + nedges]
                dloc = np.repeat(
                    np.arange(ndst), counts[bstart : bstart + ndst]
                ).astype(np.int64)
                isa = esrc < cfg.split
                for half, mask in ((0, isa), (1, ~isa)):
                    hsrc = esrc[mask]
                    hloc = dloc[mask]
                    sl = np.arange(len(hsrc))
                    t = sl // P  # tile within half (0..tpa-1)
                    p = sl % P
                    s_arr[ci, bi, half * cfg.tpa + t, p, hloc] = 1.0
                    dloc_b[
                        ci, gi, p, bj * cfg.tpb + half * cfg.tpa + t
                    ] = hloc.astype(BF16)
                    st2[
                        ci, gi, hloc,
                        (bj * cfg.tpb + half * cfg.tpa + t) * P + p,
                    ] = 1.0
                    fp = (bj * cfg.tpa + t) * P + p
                    loc = hsrc - (0 if half == 0 else cfg.split)
                    (idxa if half == 0 else idxb)[ci, gi, fp] = loc
            gdl[
                ci, gi, bj * cfg.sw : bj * cfg.sw + ndst
            ] = np.arange(bstart - d0c, bstart - d0c + ndst)

    def wrap16(a):
        # (n_cores, nbg, nia) -> (n_cores, nbg, 128, nia // 16) int16
        w = a.reshape(cfg.n_cores, nbg, nia // 16, 16).transpose(0, 1, 3, 2)
        return np.ascontiguousarray(np.tile(w, (1, 1, 8, 1)))

    idxa = wrap16(idxa)
    idxb = wrap16(idxb)
    # dst-local row per er slot, grouped by PAIRS of groups (2*bg*sw rows);
    # odd nbg gets a padded phantom group in the last pair
    npp = cfg.bg * cfg.sw
    npair = (nbg + 1) // 2
    if nbg % 2:
        gdl = np.concatenate(
            [gdl, np.full((cfg.n_cores, 1, npp), trash, dtype=np.int16)], axis=1
        )
    gdl = gdl.reshape(cfg.n_cores, npair, 2 * npp)

    # S to SBUF layout: (n_cores, nbg, P, bg*tpb*sw), fp8
    s_arr = (
        s_arr.reshape(cfg.n_cores, nbg, cfg.bg, cfg.tpb, P, cfg.sw)
        .transpose(0, 1, 4, 2, 3, 5)
        .reshape(cfg.n_cores, nbg, P, cfg.bg * cfg.tpb * cfg.sw)
    )
    st2 = st2.astype(FP8)
    meta = dict(bounds=bounds, nb=nb, nbg=nbg, nb_real=nb_real, blocks=per_core_blocks)
    return idxa, idxb, dloc_b, st2, gdl, meta


# ----------------------------------------------------------------------------
# Device program
# ----------------------------------------------------------------------------

def build_program(cfg, nb, nbg, nb_real=None):
    if nb_real is None:
        nb_real = nb
    kh, c, kc, zgc = cfg.kh, cfg.c, cfg.kc, cfg.zgc
    sw, tpb, tpa, bg, sub = cfg.sw, cfg.tpb, cfg.tpa, cfg.bg, cfg.sub
    split, rng_pad = cfg.split, cfg.rng_pad
    gpt = bg * tpb  # tiles per group (32)
    spt = sub * tpb  # tiles per sub-batch (16)
    nsub = bg // sub  # sub-batches per group
    npair = (nbg + 1) // 2
    nia = bg * tpa * P  # idxs per half-group (2048)
    npp = bg * sw  # er rows per group (256)
    bf = mybir.dt.bfloat16
    f32 = mybir.dt.float32
    fp8 = mybir.dt.float8e4
    i16 = mybir.dt.int16

    ncore = DBG_CORES or cfg.n_cores
    nc = bacc.Bacc(
        "TRN2",
        target_bir_lowering=False,
        debug=False,
        num_devices=ncore,
        dynamic_dma_scratch_size=K_SCRATCH,
        num_swdge_queues=K_NQ,
    )

    fsA_d = nc.dram_tensor("fsA", [split, cfg.d_in], bf, kind="ExternalInput")
    fsB_d = nc.dram_tensor("fsB", [cfg.n_nodes - split, cfg.d_in], bf, kind="ExternalInput")
    wz_d = nc.dram_tensor("wz", [P, kc, zgc], bf, kind="ExternalInput")
    wer_d = nc.dram_tensor("wer", [P, kc, kh], bf, kind="ExternalInput")
    crep_d = nc.dram_tensor("crep", [P, kh], bf, kind="ExternalInput")
    brep_d = nc.dram_tensor("brep", [sw, c], bf, kind="ExternalInput")
    fdp_d = nc.dram_tensor("fdp", [npair, P, kc, 2 * npp], fp8, kind="ExternalInput")
    idxa_d = nc.dram_tensor("idxa", [nbg, P, nia // 16], i16, kind="ExternalInput")
    idxb_d = nc.dram_tensor("idxb", [nbg, P, nia // 16], i16, kind="ExternalInput")
    dlc_d = nc.dram_tensor("dlc", [nbg, P, gpt], bf, kind="ExternalInput")
    iota_d = nc.dram_tensor("iota", [P, sw], bf, kind="ExternalInput")
    st2_d = nc.dram_tensor("st2", [nbg, sw, bg * tpb * P], fp8, kind="ExternalInput")

    out_d = nc.dram_tensor("out", [nb * sw, c], bf, kind="ExternalOutput")

    with tile.TileContext(nc) as tc:
        with tc.tile_pool(name="consts", bufs=1) as cpool:
            wz_sb = cpool.tile([P, kc, zgc], bf, name="wz_sb")
            nc.scalar.dma_start(out=wz_sb[:], in_=wz_d[:, :, :])
            wer_sb = cpool.tile([P, kc, kh], bf, name="wer_sb")
            nc.scalar.dma_start(out=wer_sb[:], in_=wer_d[:, :, :])
            crep_sb = cpool.tile([P, kh], bf, name="crep_sb")
            nc.scalar.dma_start(out=crep_sb[:], in_=crep_d[:, :])
            brep_sb = cpool.tile([sw, c], bf, name="brep_sb")
            nc.scalar.dma_start(out=brep_sb[:], in_=brep_d[:, :])
            iota_sb = cpool.tile([P, sw], bf, name="iota_sb")
            nc.scalar.dma_start(out=iota_sb[:], in_=iota_d[:, :])

            # ---------------- Edge processing ----------------------
            with (
                tc.tile_pool(name="edge", bufs=2) as epool,
                tc.tile_pool(name="gat", bufs=3) as gpool,
                tc.tile_pool(name="inp", bufs=3) as ipool,
                tc.tile_pool(name="hs0", bufs=1, space="PSUM") as hpool0,
                tc.tile_pool(name="hs1", bufs=1, space="PSUM") as hpool1,
                tc.tile_pool(name="hs2", bufs=1, space="PSUM") as hpool2,
                tc.tile_pool(name="hs3", bufs=1, space="PSUM") as hpool3,
                tc.tile_pool(name="elps", bufs=2, space="PSUM") as elpool,
                tc.tile_pool(name="pbps", bufs=1, space="PSUM") as pbpool,
                tc.tile_pool(name="erbps", bufs=1, space="PSUM") as erbpool,
            ):
                hpools = (hpool0, hpool1, hpool2, hpool3)
                ngr = DBG_NBG or nbg
                # the last group only processes the blocks that can be real
                # on ANY core; trailing all-pad blocks are statically skipped
                bgl_last = max(1, nb_real - (ngr - 1) * bg)
                for g in range(ngr):
                    bgl = bgl_last if g == ngr - 1 else bg
                    # gather chunks covering bgl blocks (<=768-idx calls)
                    rem = bgl * tpa * P
                    gsubs = []
                    while rem > 0:
                        gsubs.append(min(768, rem))
                        rem -= gsubs[-1]
                    gsubs = tuple(gsubs)
                    ia = ipool.tile([P, nia // 16], i16, name="ia", tag="ia")
                    nc.sync.dma_start(out=ia[:], in_=idxa_d[g])
                    ib = ipool.tile([P, nia // 16], i16, name="ib", tag="ib")
                    nc.sync.dma_start(out=ib[:], in_=idxb_d[g])
                    dlc = ipool.tile([P, gpt], bf, name="dlc", tag="dlc")
                    nc.sync.dma_start(out=dlc[:], in_=dlc_d[g])
                    # one-hot S built on DVE: S[p, tile, d] = (dloc == d)
                    ssb = epool.tile([P, gpt, sw], bf, name="ssb", tag="ssb")
                    nc.vector.tensor_tensor(
                        out=ssb[:, :, :],
                        in0=bass.AP(
                            dlc.tensor, dlc.offset, [[gpt, P], [1, gpt], [0, sw]]
                        ),
                        in1=bass.AP(
                            iota_sb.tensor, iota_sb.offset,
                            [[sw, P], [0, gpt], [1, sw]],
                        ),
                        op=mybir.AluOpType.is_equal,
                    )
                    st2sb = ipool.tile(
                        [sw, bg * tpb * P], fp8, name="st2sb", tag="st2sb"
                    )
                    nc.sync.dma_start(out=st2sb[:], in_=st2_d[g])

                    # transposed MoE gathers: [din%128, din//128, slot]
                    subs = gsubs
                    offs = tuple(sum(gsubs[:i]) for i in range(len(gsubs)))
                    gA = []
                    gB = []
                    # interleave A/B chunks so the first sub-batch's inputs
                    # (early chunks of BOTH halves) arrive soonest
                    for ci_, (ou, su) in enumerate(zip(offs, subs)):
                        for gl, nm, src_ap, ixt in (
                            (gA, "gA", fsA_d, ia), (gB, "gB", fsB_d, ib)
                        ):
                            gt = gpool.tile(
                                [P, kc * su], bf,
                                name=f"{nm}{ci_}", tag=f"{nm}{ci_}",
                            )
                            gl.append(gt)
                            nc.gpsimd.dma_gather(
                                out_ap=bass.AP(
                                    gt.tensor,
                                    gt.offset,
                                    [[kc * su, P], [su, kc], [1, su]],
                                ),
                                in_ap=src_ap[:, :],
                                idxs_ap=ixt[:, ou // 16 : (ou + su) // 16],
                                num_idxs=su,
                                num_idxs_reg=su,
                                elem_size=cfg.d_in,
                                transpose=True,
                                queue_num=ci_ % K_NQ,
                            )

                    # er rows for the group pair: one 512-idx transposed
                    # gather of dst feature rows from the per-core local
                    # table, then a tiny GEMM against W_er per block.
                    if g % 2 == 0:
                        gd = epool.tile([P, kc, 2 * npp], fp8, name="gd", tag="gd")
                        nc.scalar.dma_start(out=gd[:], in_=fdp_d[g // 2])
                        erb_ps = erbpool.tile(
                            [sw, 2 * bg, kh], f32, name="erb_ps", tag="erb_ps"
                        )
                        for b in range(2 * bg):
                            for k in range(kc):
                                nc.tensor.matmul(
                                    erb_ps[:, b, :],
                                    lhsT=gd[:, k, b * sw : (b + 1) * sw],
                                    rhs=wer_sb[:, k, :],
                                    start=(k == 0),
                                    stop=(k == kc - 1),
                                )
                        # fold in the constant c = c_el + c_er while moving to
                        # SBUF (so the erg expansion matmul emits er + c)
                        erbs = epool.tile([sw, 2 * bg, kh], bf, name="erbs", tag="erbs")
                        nc.vector.tensor_tensor(
                            out=erbs[:, :, :],
                            in0=erb_ps[:, :, :],
                            in1=bass.AP(
                                crep_sb.tensor, crep_sb.offset,
                                [[kh, sw], [0, 2 * bg], [1, kh]],
                            ),
                            op=mybir.AluOpType.add,
                        )
                    go = (g % 2) * bg

                    stg = epool.tile([sw, bg, zgc], bf, name="stg", tag="stg")
                    # normal groups: 2 sub-batches of 4 blocks; the LAST group
                    # runs 4 sub-batches of 2 blocks on alternating PSUM pool
                    # pairs so its drain chain pipelines (shorter tail).
                    last = g == ngr - 1
                    if last:
                        plan = [
                            (b0, min(2, bgl - b0), (i % 2) * 2)
                            for i, b0 in enumerate(range(0, bgl, 2))
                        ]
                    else:
                        plan = [(q * sub, sub, 0) for q in range(nsub)]
                    for qi, (b0, sub_g, pbase) in enumerate(plan):
                        mf = min(K_MF, sub_g - 1) if sub_g > 1 else K_MF
                        # hs GEMM per block into resident 1-bank PSUM tiles;
                        # el GEMM + er one-hot expansion accumulate into a
                        # shared el PSUM tile: x = el + er + c.
                        elps = elpool.tile([P, spt, kh], f32, name="elps", tag="elps")
                        hsps = []
                        for j in range(sub_g):
                            bj = b0 + j
                            hsp = hpools[pbase + j].tile(
                                [P, tpb, c], f32,
                                name=f"hs{pbase + j}", tag=f"hs{pbase + j}",
                            )
                            hsps.append(hsp)
                            for half, gl in ((0, gA), (1, gB)):
                                for t in range(tpa):
                                    col = (bj * tpa + t) * P
                                    ui = next(
                                        i for i in range(len(subs))
                                        if offs[i] <= col < offs[i] + subs[i]
                                    )
                                    ou, su = offs[ui], subs[ui]
                                    gt = gl[ui]
                                    lhs = bass.AP(
                                        gt.tensor,
                                        gt.offset + (col - ou),
                                        [[kc * su, P], [su, kc], [1, P]],
                                    )
                                    tt = half * tpa + t
                                    for k in range(kc):
                                        nc.tensor.matmul(
                                            hsp[:, tt, :],
                                            lhsT=lhs[:, k, :],
                                            rhs=wz_sb[:, k, 0:c],
                                            start=(k == 0),
                                            stop=(k == kc - 1),
                                        )
                                    for k in range(kc):
                                        nc.tensor.matmul(
                                            elps[:, j * tpb + tt, :],
                                            lhsT=lhs[:, k, :],
                                            rhs=wz_sb[:, k, c:zgc],
                                            start=(k == 0),
                                            stop=False,
                                        )
                                    # er expansion accumulates er + c on top
                                    nc.tensor.matmul(
                                        elps[:, j * tpb + tt, :],
                                        lhsT=st2sb[
                                            :, (bj * tpb + tt) * P
                                            : (bj * tpb + tt + 1) * P
                                        ],
                                        rhs=erbs[:, go + bj, :],
                                        start=False,
                                        stop=True,
                                    )

                        spt_g = sub_g * tpb
                        # w = max(exp(x), exp(0.2x)); exps read el PSUM
                        e1 = epool.tile([P, spt, kh], bf, name="e1", tag="e1")
                        nc.scalar.activation(
                            e1[:, 0:spt_g, :], elps[:, 0:spt_g, :],
                            mybir.ActivationFunctionType.Exp,
                        )
                        e2 = epool.tile([P, spt, kh], bf, name="e2", tag="e2")
                        nc.scalar.activation(
                            e2[:, 0:spt_g, :], elps[:, 0:spt_g, :],
                            mybir.ActivationFunctionType.Exp,
                            scale=float(cfg.neg_slope),
                        )
                        zsb = epool.tile(
                            [P, spt, zgc], bf, name=f"zsb{qi % 2}", tag=f"zsb{qi % 2}"
                        )
                        w_ap = bass.AP(
                            zsb.tensor, zsb.offset + c,
                            [[spt * zgc, P], [zgc, spt_g], [1, kh]],
                        )
                        nc.vector.tensor_tensor(
                            out=w_ap, in0=e1[:, 0:spt_g, :], in1=e2[:, 0:spt_g, :],
                            op=mybir.AluOpType.max,
                        )
                        # hybrid PSUM drain: first mf blocks get the merged
                        # DVE move+multiply from PSUM; the rest are plain Act
                        # copies followed by one in-place bf16 2x-rate DVE
                        # multiply over their zsb range.
                        for j in range(mf, sub_g):
                            nc.scalar.copy(
                                bass.AP(
                                    zsb.tensor, zsb.offset + j * tpb * zgc,
                                    [[spt * zgc, P], [zgc, tpb], [1, c]],
                                ),
                                hsps[j][:, :, :],
                            )
                        for j in range(mf):
                            hs_out = bass.AP(
                                zsb.tensor, zsb.offset + j * tpb * zgc,
                                [[spt * zgc, P], [zgc, tpb], [kh, cfg.dh], [1, kh]],
                            )
                            hs_in = bass.AP(
                                hsps[j].tensor, hsps[j].offset,
                                [[tpb * c, P], [c, tpb], [kh, cfg.dh], [1, kh]],
                            )
                            wj_ap = bass.AP(
                                zsb.tensor, zsb.offset + j * tpb * zgc + c,
                                [[spt * zgc, P], [zgc, tpb], [0, cfg.dh], [1, kh]],
                            )
                            nc.vector.tensor_tensor(
                                out=hs_out, in0=hs_in, in1=wj_ap,
                                op=mybir.AluOpType.mult,
                            )
                        if mf < sub_g:
                            rng = bass.AP(
                                zsb.tensor, zsb.offset + mf * tpb * zgc,
                                [[spt * zgc, P], [zgc, (sub_g - mf) * tpb],
                                 [kh, cfg.dh], [1, kh]],
                            )
                            wr_ap = bass.AP(
                                zsb.tensor, zsb.offset + mf * tpb * zgc + c,
                                [[spt * zgc, P], [zgc, (sub_g - mf) * tpb],
                                 [0, cfg.dh], [1, kh]],
                            )
                            nc.vector.tensor_tensor(
                                out=rng, in0=rng, in1=wr_ap,
                                op=mybir.AluOpType.mult,
                            )

                        # segment-sum matmuls + one flush per sub-batch
                        pb = pbpool.tile([sw, sub, zgc], f32, name="pb", tag="pb")
                        for j in range(sub_g):
                            bj = b0 + j
                            for t in range(tpb):
                                nc.tensor.matmul(
                                    pb[:, j, :],
                                    lhsT=ssb[:, bj * tpb + t, :],
                                    rhs=zsb[:, j * tpb + t, :],
                                    start=(t == 0),
                                    stop=(t == tpb - 1),
                                )
                        nc.scalar.copy(
                            stg[:, b0 : b0 + sub_g, :], pb[:, 0:sub_g, :]
                        )

                    # normalize + bias + flush (per group; the last group
                    # normalizes per half so the final chain is shorter)
                    lp = nc.allow_low_precision(reason="bf16 softmax denominators")
                    lp.__enter__()
                    st_t, st_off = stg.tensor, stg.offset
                    outp = epool.tile([sw, bg, c], bf, name="outp", tag="outp")
                    o_t, o_off = outp.tensor, outp.offset
                    rcp = epool.tile([sw, bg * kh], bf, name="rcp", tag="rcp")
                    halves = (
                        tuple((b0, min(2, bgl - b0)) for b0 in range(0, bgl, 2))
                        if last else ((0, bg),)
                    )
                    for bo, nblk in halves:
                        den = bass.AP(
                            st_t, st_off + bo * zgc + c,
                            [[bg * zgc, sw], [zgc, nblk], [1, kh]],
                        )
                        nc.vector.tensor_scalar_max(den, den, 1e-20)
                        rcps = bass.AP(
                            rcp.tensor, rcp.offset + bo * kh,
                            [[bg * kh, sw], [kh, nblk], [1, kh]],
                        )
                        nc.vector.reciprocal(rcps, den)
                        num_ap = bass.AP(
                            st_t, st_off + bo * zgc,
                            [[bg * zgc, sw], [zgc, nblk], [kh, cfg.dh], [1, kh]],
                        )
                        out_ap = bass.AP(
                            o_t, o_off + bo * c,
                            [[bg * c, sw], [c, nblk], [kh, cfg.dh], [1, kh]],
                        )
                        rcp_ap = bass.AP(
                            rcp.tensor, rcp.offset + bo * kh,
                            [[bg * kh, sw], [kh, nblk], [0, cfg.dh], [1, kh]],
                        )
                        nc.vector.tensor_tensor(
                            out=out_ap, in0=num_ap, in1=rcp_ap,
                            op=mybir.AluOpType.mult,
                        )
                        b_ap = bass.AP(
                            brep_sb.tensor, brep_sb.offset,
                            [[c, sw], [0, nblk], [1, c]],
                        )
                        ofl = bass.AP(
                            o_t, o_off + bo * c, [[bg * c, sw], [c, nblk], [1, c]]
                        )
                        nc.vector.tensor_tensor(
                            out=ofl, in0=ofl, in1=b_ap, op=mybir.AluOpType.add
                        )
                        nc.sync.dma_start(
                            out=bass.AP(
                                out_d.ap().tensor,
                                (g * bg + bo) * sw * c,
                                [[c, sw], [sw * c, nblk], [1, c]],
                            ),
                            in_=outp[:, bo : bo + nblk, :],
                        )
                    lp.__exit__(None, None, None)

    nc.compile()
    return nc


# ----------------------------------------------------------------------------
# Entry point
# ----------------------------------------------------------------------------

def _run(cfg, inputs, trace=False):
    feat_src = np.asarray(inputs["feat_src"], dtype=np.float32)
    feat_dst = np.asarray(inputs["feat_dst"], dtype=np.float32)
    W_src = np.asarray(inputs["W_src"], dtype=np.float32)
    b_src = np.asarray(inputs["b_src"], dtype=np.float32)
    W_dst = np.asarray(inputs["W_dst"], dtype=np.float32)
    b_dst = np.asarray(inputs["b_dst"], dtype=np.float32)
    attn = np.asarray(inputs["attn"], dtype=np.float32)
    src_idx = np.asarray(inputs["src_idx"]).astype(np.int64)
    dst_idx = np.asarray(inputs["dst_idx"]).astype(np.int64)

    Wz, Wer, c_rep, b_rep = build_weights(cfg, W_src, b_src, W_dst, b_dst, attn)
    idxa, idxb, dloc_b, st2, gdl_rows, meta = build_schedule(cfg, src_idx, dst_idx)
    nb, nbg, bounds = meta["nb"], meta["nbg"], meta["bounds"]

    feat_bf = feat_src.astype(BF16)
    fsA = np.ascontiguousarray(feat_bf[: cfg.split])
    fsB = np.ascontiguousarray(feat_bf[cfg.split :])

    fdloc = np.zeros((cfg.n_cores, cfg.rng_pad, cfg.d_in), dtype=BF16)
    for ci in range(cfg.n_cores):
        d0, d1 = bounds[ci], bounds[ci + 1]
        fdloc[ci, : d1 - d0] = feat_dst[d0:d1].astype(BF16)

    # host-side pre-transposed er feature rows per group-pair:
    # fdp[pair, din%128, k, row] = fdloc[gdl[pair, row], k*128 + din%128]
    npp2 = 2 * cfg.bg * cfg.sw
    npair = (nbg + 1) // 2
    fdp = np.zeros((cfg.n_cores, npair, P, cfg.kc, npp2), dtype=FP8)
    for ci in range(cfg.n_cores):
        rows = gdl_rows[ci]  # (npair, npp2) local dst row per slot
        sel = fdloc[ci][rows].astype(FP8)  # (npair, npp2, d_in)
        fdp[ci] = sel.reshape(npair, npp2, cfg.kc, P).transpose(0, 3, 2, 1)

    iota_rep = np.tile(
        np.arange(cfg.sw, dtype=np.float32)[None, :], (P, 1)
    ).astype(BF16)

    nc = build_program(cfg, nb, nbg, meta["nb_real"])

    in_maps = []
    for ci in range(cfg.n_cores):
        in_maps.append(
            {
                "fsA": fsA,
                "fsB": fsB,
                "wz": Wz,
                "wer": Wer,
                "crep": c_rep,
                "brep": b_rep,
                "fdp": fdp[ci],
                "idxa": idxa[ci],
                "idxb": idxb[ci],
                "dlc": dloc_b[ci],
                "iota": iota_rep,
                "st2": st2[ci],
            }
        )

    ncore = DBG_CORES or cfg.n_cores
    res = run_bass_kernel_spmd(
        nc, in_maps[:ncore], core_ids=list(range(ncore)), trace=trace
    )

    perm = _head_minor_perm(cfg)
    out = np.zeros((cfg.n_nodes, cfg.c), dtype=np.float32)
    for ci in range(DBG_CORES or cfg.n_cores):
        tab = np.asarray(res.results[ci]["out"]).astype(np.float32)  # [nb*sw, c]
        for bi, (bstart, ndst, _) in enumerate(meta["blocks"][ci]):
            out[bstart : bstart + ndst, perm] = tab[bi * cfg.sw : bi * cfg.sw + ndst]
    deg = np.bincount(dst_idx, minlength=cfg.n_nodes)
    out[deg == 0] = 0.0
    return out, res


def kernel(**inputs) -> np.ndarray:
    out, _ = _run(FULL_CFG, inputs, trace=False)
    return out
